# revision 5
# baseline (speedup 1.0000x reference)
"""Trainium2 Bass kernel for nn_CausalAttentionLayer (sparse_attention).

Reference computes, per batch b (B=32, Nq=Nk=1024, C=128, CM=256):
    S = Q @ K^T                      # [1024, 1024], no 1/sqrt(d) scale
    P = softmax(S, axis=-1) * strict_lower_mask   # mask AFTER full-row softmax
    O = P @ V                        # [1024, 256]

Sharding: data-parallel over batch, 4 batches per core on 8 NeuronCores.

Device algorithm (per batch), in the transposed S^T = K Q^T layout (k on
partitions, q on free) so no on-device transposes are needed:
  - S^T block = Kh^T Qh (bf16 matmul) + one fp8e4m3 DoubleRow matmul that
    computes K^T Ql + Kl^T Q in a single pass at 0.5 cycles/row (two K=128
    contractions packed per instruction; operands are scale-balanced by
    2^+-4 so products land in true units). S error ~5e-3 abs, final output
    rel err ~2.5e-3 (vs 2e-2 budget).
  - P^T = exp(S^T) on ScalarE in [128, 1024] tiles, written bf16 to SBUF
  - l[q] = sum_k exp(S[q, k]) over ALL k: pt tiles pair-summed on DVE/
    GpSimd into 4 f32r partials, ones-matmuls accumulate them to a
    [1, 1024] PSUM row (lagged so the PE never waits), ScalarE copy out,
    partition-spread via 8 small DMAs, reciprocal on DVE.
  - strict-upper mask on diagonal blocks (DVE, bf16 2x mode), causal PV
    (36 of 64 block mms, bf16 moving V), O *= r (per-partition scalar),
    DMA out.

Scheduling: the PV matmul groups of batch b-1 are interleaved INTO the
S-phase emission of batch b (engines execute their queues in order), so
the PE always has streaming work while ScalarE exp and the l-chain of the
current batch catch up. PE work is ~10.7us/batch and is the target pole.
"""

import sys
from contextlib import ExitStack

import numpy as np

sys.path.insert(0, "/opt/trn_rl_repo")

import ml_dtypes  # noqa: E402

import concourse.tile as tile  # noqa: E402
from concourse import bacc, mybir  # noqa: E402
from concourse.bass_utils import run_bass_kernel_spmd  # noqa: E402

N_CORES = 8
B_TOTAL = 32
NQ = 1024
C = 128
CM = 256
NBLK = NQ // 128  # 8
FP8_SCALE = 16.0  # 2^4 balance factor for the DoubleRow cross terms

_cache = {}

f32 = mybir.dt.float32
f32r = mybir.dt.float32r
bf16 = mybir.dt.bfloat16
f8e4 = mybir.dt.float8e4
DR = mybir.MatmulPerfMode.DoubleRow


def emit_kernel(nc, tc, ctx, aps, b_core):
    qhk, qx, kx, v, mask, ones, out = aps
    const_pool = ctx.enter_context(tc.tile_pool(name="const", bufs=1))
    qk_pool = ctx.enter_context(tc.tile_pool(name="qk", bufs=2))
    qx_pool = ctx.enter_context(tc.tile_pool(name="qx", bufs=2))
    kx_pool = ctx.enter_context(tc.tile_pool(name="kx", bufs=2))
    v_pool = ctx.enter_context(tc.tile_pool(name="vp", bufs=2))
    pt_pool = ctx.enter_context(tc.tile_pool(name="pt", bufs=16))
    ptm_pool = ctx.enter_context(tc.tile_pool(name="ptm", bufs=2))
    acc_pool = ctx.enter_context(tc.tile_pool(name="acc", bufs=8))
    l_pool = ctx.enter_context(tc.tile_pool(name="lsb", bufs=2))
    r_pool = ctx.enter_context(tc.tile_pool(name="rsb", bufs=2))
    o_pool = ctx.enter_context(tc.tile_pool(name="osb", bufs=2))
    ps_s = ctx.enter_context(tc.tile_pool(name="ps_s", bufs=2, space="PSUM"))
    ps_o = ctx.enter_context(tc.tile_pool(name="ps_o", bufs=2, space="PSUM"))
    ps_l = ctx.enter_context(tc.tile_pool(name="ps_l", bufs=1, space="PSUM"))

    mask_sb = const_pool.tile([128, 128], bf16)
    nc.sync.dma_start(mask_sb[:], mask)
    ones_sb = const_pool.tile([128, 1], f32r)
    nc.sync.dma_start(ones_sb[:], ones)

    def emit_pv_group(st, j):
        # one PV output block: O[128q, 256] = sum_{i<=j} P^T_i[:, jslc]^T V_i
        jslc = slice(128 * j, 128 * (j + 1))
        o_ps = ps_o.tile([128, CM], f32, tag="o", name="o_ps")
        for i in range(j + 1):
            lhsT = st["ptm"][:, jslc] if i == j else st["pt"][i][:, jslc]
            nc.tensor.matmul(
                o_ps[:], lhsT, st["v"][:, i], start=(i == 0), stop=(i == j)
            )
        nc.vector.tensor_scalar_mul(
            st["o_all"][:, CM * j : CM * (j + 1)], o_ps[:],
            st["r_sb"][:, j : j + 1],
        )
        nc.sync.dma_start(
            out[st["b"], jslc, :], st["o_all"][:, CM * j : CM * (j + 1)]
        )

    def emit_pv_head(st):
        # l [1, 1024] -> [128, 8] partition-spread via 8 small DMAs
        # (lt[p, j] = l[128j + p]), then reciprocal on DVE
        lt_sb = r_pool.tile([128, NBLK], f32, tag="lt", name="lt_sb")
        for j in range(NBLK):
            nc.sync.dma_start(
                lt_sb[:, j : j + 1],
                st["l_sb"][0:1, 128 * j : 128 * (j + 1)],
            )
        r_sb = r_pool.tile([128, NBLK], f32, tag="r", name="r_sb")
        nc.vector.reciprocal(r_sb[:], lt_sb[:])
        st["r_sb"] = r_sb
        st["o_all"] = o_pool.tile(
            [128, NBLK * CM], f32, tag="o_sb", name="o_all"
        )

    def emit_s_phase(b, prev):
        st = {"b": b}
        qhk_sb = qk_pool.tile([C, 2 * NQ], bf16, tag="qhk", name="qhk_sb")
        for half in (0, 1):  # qh first (first matmul's moving input)
            nc.sync.dma_start(
                qhk_sb[:, NQ * half : NQ * (half + 1)],
                qhk[b, :, NQ * half : NQ * (half + 1)],
            )
        qx_sb = qx_pool.tile([C, 2, 2, 512], f8e4, tag="qx", name="qx_sb")
        for h in (0, 1):
            nc.sync.dma_start(qx_sb[:, h], qx[b, :, NQ * h : NQ * (h + 1)])
        kx_sb = kx_pool.tile(
            [C, NBLK, 2, 128], f8e4, tag="kx", name="kx_sb"
        )
        for half in (0, 1):
            nc.sync.dma_start(
                kx_sb[:, 4 * half : 4 * (half + 1)],
                kx[b, :, NQ * half : NQ * (half + 1)],
            )
        # V loaded as 8 per-block DMAs (queue parallelism); used one batch
        # later by PV, so plenty of prefetch slack
        v_sb = v_pool.tile([128, NBLK, CM], bf16, tag="v", name="v_sb")
        for i in range(NBLK):
            nc.sync.dma_start(v_sb[:, i], v[b, 128 * i : 128 * (i + 1), :])
        st["v"] = v_sb
        st["pt"] = []
        st["a1"] = []
        st["ptm"] = ptm_pool.tile([128, NQ], bf16, tag="ptm", name="ptm")
        psl = ps_l.tile([1, NQ], f32, tag="psl", name="psl")
        st["psl"] = psl

        def pair_add(eng, p):
            a = acc_pool.tile([128, NQ], f32r, tag="acc", name="acc")
            eng.tensor_add(a[:], st["pt"][2 * p][:], st["pt"][2 * p + 1][:])
            st["a1"].append(a)

        def l_mms(p):
            for h in (0, 1):
                nc.tensor.matmul(
                    psl[:, 512 * h : 512 * (h + 1)],
                    ones_sb[:],
                    st["a1"][p][:, 512 * h : 512 * (h + 1)],
                    start=(p == 0),
                    stop=(p == 3),
                    skip_group_check=True,
                )

        for i in range(NBLK):
            kh_blk = qhk_sb[:, NQ + 128 * i : NQ + 128 * (i + 1)]
            s_ps = ps_s.tile([128, NQ], f32, tag="s", name="s_ps")
            for h in (0, 1):
                qslc = slice(512 * h, 512 * (h + 1))
                nc.tensor.matmul(
                    s_ps[:, qslc], kh_blk, qhk_sb[:, qslc],
                    start=True, stop=False, skip_group_check=True,
                )
                nc.tensor.matmul(
                    s_ps[:, qslc], kx_sb[:, i], qx_sb[:, h],
                    perf_mode=DR, start=False, stop=True,
                    skip_group_check=True,
                )
            pt_i = pt_pool.tile([128, NQ], bf16, tag="pt", name="pt_i")
            nc.scalar.activation(
                pt_i[:], s_ps[:], mybir.ActivationFunctionType.Exp
            )
            st["pt"].append(pt_i)
            # strict-upper mask of the diagonal block, as soon as available
            kslc = slice(128 * i, 128 * (i + 1))
            nc.vector.tensor_mul(
                st["ptm"][:, kslc], pt_i[:, kslc], mask_sb[:]
            )
            # lagged l partial sums on DVE / GpSimd + lagged ones-matmuls
            if i == 2:
                pair_add(nc.vector, 0)
            elif i == 4:
                pair_add(nc.gpsimd, 1)
                l_mms(0)
            elif i == 6:
                pair_add(nc.vector, 2)
                l_mms(1)
            # interleave the previous batch's PV groups so the PE always
            # has streaming work while ScalarE catches up on exp
            if prev is not None and i >= 1:
                if i == 1:
                    emit_pv_head(prev)
                emit_pv_group(prev, NBLK - i)
        if prev is not None:
            emit_pv_group(prev, 0)
        return st

    def emit_l_tail(st):
        # final partial + ones-mms + l copy: by the time the PE drains the
        # interleaved S+PV stream (~9.7us), a1[3] (~9.3us) is ready
        a = acc_pool.tile([128, NQ], f32r, tag="acc", name="acc_t")
        nc.gpsimd.tensor_add(a[:], st["pt"][6][:], st["pt"][7][:])
        st["a1"].append(a)
        for p in (2, 3):
            for h in (0, 1):
                nc.tensor.matmul(
                    st["psl"][:, 512 * h : 512 * (h + 1)],
                    ones_sb[:],
                    st["a1"][p][:, 512 * h : 512 * (h + 1)],
                    start=False,
                    stop=(p == 3),
                    skip_group_check=True,
                )
        l_sb = l_pool.tile([1, NQ], f32, tag="l", name="l_sb")
        nc.scalar.copy(l_sb[:], st["psl"][:])
        st["l_sb"] = l_sb

    prev = None
    for b in range(b_core):
        st = emit_s_phase(b, prev)
        emit_l_tail(st)
        prev = st
    # trailing PV for the last batch (no S phase left to interleave with;
    # under the For_i timing loop the next iteration's S phase follows it
    # back-to-back in the PE queue)
    emit_pv_head(prev)
    for j in reversed(range(NBLK)):
        emit_pv_group(prev, j)


def declare_io(nc, b_core):
    qhk = nc.dram_tensor(
        "qhk", [b_core, C, 2 * NQ], bf16, kind="ExternalInput"
    ).ap()
    qx = nc.dram_tensor(
        "qx", [b_core, C, 2 * NQ], f8e4, kind="ExternalInput"
    ).ap()
    kx = nc.dram_tensor(
        "kx", [b_core, C, 2 * NQ], f8e4, kind="ExternalInput"
    ).ap()
    v = nc.dram_tensor("v", [b_core, NQ, CM], bf16, kind="ExternalInput").ap()
    mask = nc.dram_tensor("mask", [128, 128], bf16, kind="ExternalInput").ap()
    ones = nc.dram_tensor("ones", [128, 1], f32r, kind="ExternalInput").ap()
    out = nc.dram_tensor("out", [b_core, NQ, CM], f32, kind="ExternalOutput").ap()
    return (qhk, qx, kx, v, mask, ones, out)


def build(b_core):
    """Build + compile the per-core Bass program processing b_core batches."""
    nc = bacc.Bacc(
        "TRN2", target_bir_lowering=False, debug=False, num_devices=N_CORES
    )
    aps = declare_io(nc, b_core)
    with tile.TileContext(nc) as tc, ExitStack() as ctx:
        emit_kernel(nc, tc, ctx, aps, b_core)

    nc.compile()
    return nc


def host_prep(query, key, value):
    """Full inputs -> per-core in_maps (host-side layout prep + sharding)."""
    q = np.ascontiguousarray(np.asarray(query, dtype=np.float32)).reshape(
        B_TOTAL, NQ, C
    )
    k = np.ascontiguousarray(np.asarray(key, dtype=np.float32)).reshape(
        B_TOTAL, NQ, C
    )
    v = np.ascontiguousarray(np.asarray(value, dtype=np.float32)).reshape(
        B_TOTAL, NQ, CM
    )
    qt = np.ascontiguousarray(q.transpose(0, 2, 1))  # [B, C, NQ]
    kt = np.ascontiguousarray(k.transpose(0, 2, 1))
    bft = ml_dtypes.bfloat16
    f8t = ml_dtypes.float8_e4m3
    qh = qt.astype(bft)
    ql = qt - qh.astype(np.float32)
    kh = kt.astype(bft)
    kl = kt - kh.astype(np.float32)

    # DoubleRow operands, scale-balanced so each packed product lands in
    # true units: stream0 = (K/16)^T (Ql*16), stream1 = (Kl*16)^T (Q/16)
    qxa = np.empty((B_TOTAL, C, 2, 2, 512), dtype=f8t)
    kxa = np.empty((B_TOTAL, C, NBLK, 2, 128), dtype=f8t)
    ql_s = (ql * FP8_SCALE).astype(f8t)
    qd_s = (qt / FP8_SCALE).astype(f8t)
    kl_s = (kl * FP8_SCALE).astype(f8t)
    kd_s = (kt / FP8_SCALE).astype(f8t)
    for h in (0, 1):
        qslc = slice(512 * h, 512 * (h + 1))
        qxa[:, :, h, 0, :] = ql_s[:, :, qslc]
        qxa[:, :, h, 1, :] = qd_s[:, :, qslc]
    for i in range(NBLK):
        kslc = slice(128 * i, 128 * (i + 1))
        kxa[:, :, i, 0, :] = kd_s[:, :, kslc]
        kxa[:, :, i, 1, :] = kl_s[:, :, kslc]

    qhk = np.ascontiguousarray(
        np.concatenate([qh, kh], axis=2)
    )  # [B, C, 2*NQ] bf16
    qxa = np.ascontiguousarray(qxa.reshape(B_TOTAL, C, 2 * NQ))
    kxa = np.ascontiguousarray(kxa.reshape(B_TOTAL, C, 2 * NQ))
    vb = np.ascontiguousarray(v.astype(bft))
    mask_np = np.triu(np.ones((128, 128), dtype=np.float32), k=1).astype(bft)
    ones_np = np.ones((128, 1), dtype=np.float32)

    b_core = B_TOTAL // N_CORES
    in_maps = []
    for cidx in range(N_CORES):
        sl = slice(b_core * cidx, b_core * (cidx + 1))
        in_maps.append(
            {
                "qhk": np.ascontiguousarray(qhk[sl]),
                "qx": np.ascontiguousarray(qxa[sl]),
                "kx": np.ascontiguousarray(kxa[sl]),
                "v": np.ascontiguousarray(vb[sl]),
                "mask": mask_np,
                "ones": ones_np,
            }
        )
    return in_maps


def kernel(query, key, value):
    b_core = B_TOTAL // N_CORES
    if "nc" not in _cache:
        _cache["nc"] = build(b_core)
    nc = _cache["nc"]
    in_maps = host_prep(query, key, value)
    res = run_bass_kernel_spmd(
        nc, in_maps, core_ids=list(range(N_CORES)), trace=False
    )
    out = np.concatenate([r["out"] for r in res.results], axis=0)
    return out.reshape(B_TOTAL, 32, 32, CM).astype(np.float32)


if __name__ == "__main__":
    rng = np.random.default_rng(0)
    q = rng.standard_normal((B_TOTAL, 32, 32, C), dtype=np.float32)
    k = rng.standard_normal((B_TOTAL, 32, 32, C), dtype=np.float32)
    v = rng.standard_normal((B_TOTAL, 32, 32, CM), dtype=np.float32)
    o = kernel(query=q, key=k, value=v)
    print(o.shape, o.dtype)
